# revision 21
# baseline (speedup 1.0000x reference)
"""Trainium2 Bass kernel for masked spatial attention softmax.

Computes S = softmax((F_a@Wq.T + bq) @ (F_s@Wk.T + bk).T / sqrt(d) + mask)
over 8 NeuronCores, data-parallel over batch.

Key algebraic restructure: QK = Q_a @ K_s.T = ((F_a@Wq.T + bq) @ Wk) @ F_s.T
+ (Q_a . bk) 1^T.  The bk term is constant along the softmax axis, so it
drops out of the softmax entirely; K_s is never materialized.  This halves
the matmul FLOPs and removes a 4096xd projection per batch.

Software pipeline: batch b's QK/exp phase interleaves batch b+1's F_s
transposes on the PE stream (keeps PE dense, HAM warm); loads prefetch
two batches ahead.
"""

import math
from contextlib import ExitStack

import numpy as np
import ml_dtypes

import concourse.bass as bass
import concourse.tile as tile
from concourse import bacc, mybir
from concourse.masks import make_identity

# Problem shapes (hardcoded per contract; spec: B=32, T=256, HW=4096, d=256)
B_FULL = 32
N_CORES = 8
BS = B_FULL // N_CORES  # batches per core
T = 256
HW = 4096
D = 256
SCALE = 1.0 / math.sqrt(D)  # 1/16
MASK_NEG = -80.0  # exp(-80 + max_logit) << 1e-30; stays in ACT exp valid range

F32 = mybir.dt.float32
BF16 = mybir.dt.bfloat16


def _build_body(tc, ctx, F_a, F_s, mbig, Wq, Wk, bq, S):
    nc = tc.nc

    singles = ctx.enter_context(tc.tile_pool(name="singles", bufs=1))
    fnat_pool = ctx.enter_context(tc.tile_pool(name="fnat", bufs=2))
    fst_pool = ctx.enter_context(tc.tile_pool(name="fst", bufs=2))
    qpool = ctx.enter_context(tc.tile_pool(name="qpool", bufs=2))
    ppool = ctx.enter_context(tc.tile_pool(name="ppool", bufs=3))
    spool = ctx.enter_context(tc.tile_pool(name="spool", bufs=2))
    stats = ctx.enter_context(tc.tile_pool(name="stats", bufs=4))
    psum_tr = ctx.enter_context(tc.tile_pool(name="psum_tr", bufs=1, space="PSUM"))
    psum_qk = ctx.enter_context(tc.tile_pool(name="psum_qk", bufs=3, space="PSUM"))
    psum_pj = ctx.enter_context(tc.tile_pool(name="psum_pj", bufs=1, space="PSUM"))

    # ---- constants ----
    ident16 = singles.tile([128, 128], BF16, tag="ident16", name="ident16")
    make_identity(nc, ident16[:])
    ones16 = singles.tile([1, 128], BF16, tag="ones16", name="ones16")
    nc.vector.memset(ones16[:], 1.0)

    # mask additive term, all batches: [1, BS*HW] bf16 (0 or MASK_NEG)
    mb_sb = singles.tile([1, BS * HW], BF16, tag="mb", name="mb")
    nc.sync.dma_start(out=mb_sb[:], in_=mbig.rearrange("b s -> (b s)")[None, :])

    # bq as per-partition scalars: [128, 2] (do-tile major in free dim)
    bq_sb = singles.tile([128, 2], F32, tag="bq", name="bq")
    nc.sync.dma_start(out=bq_sb[:], in_=bq.rearrange("(a p) -> p a", p=128))

    # Wk natural: lhsT[j, i] for Q~T = sum_j Wk[j,i] Q.T[j,t]  (bf16 cast DMA)
    wk_sb = singles.tile([128, 2, D], BF16, tag="wk", name="wk")
    nc.gpsimd.dma_start(out=wk_sb[:], in_=Wk.rearrange("(jh jl) i -> jl jh i", jl=128))

    # Wq loaded natural (bf16), then PE-transposed to WqT[di, do]
    wq_nat = singles.tile([128, 2, D], BF16, tag="wqn", name="wqn")
    nc.gpsimd.dma_start(
        out=wq_nat[:], in_=Wq.rearrange("(oh ol) i -> ol oh i", ol=128)
    )
    wqt = singles.tile([128, 2, D], BF16, tag="wqt", name="wqt")
    for k in range(2):  # di tile
        pj = psum_pj.tile([128, D], BF16, tag="pj", name="pj")
        for m in range(2):  # do tile
            nc.tensor.matmul(
                pj[:, m * 128:(m + 1) * 128],
                wq_nat[:, m, k * 128:(k + 1) * 128],
                ident16[:],
                is_transpose=True,
                start=(m == 0),
                stop=(m == 1),
            )
        nc.vector.tensor_copy(out=wqt[:, k, :], in_=pj[:])

    fa_t, fnat_t, fst_t, qct_t = {}, {}, {}, {}

    def load_batch(b):
        """Prefetch F_a[b] (small, first) and F_s[b] in halves (SWDGE casts)."""
        fa = qpool.tile([128, 2, D], BF16, tag="fa", name="fa")  # [tl, th, d]
        nc.gpsimd.dma_start(
            out=fa[:], in_=F_a[b].rearrange("(th tl) d -> tl th d", tl=128)
        )
        fa_t[b] = fa
        fnat = fnat_pool.tile([128, 32, D], BF16, tag="fnat", name="fnat")  # [sl, sh, c]
        fsrc = F_s[b].rearrange("(sh sl) c -> sl sh c", sl=128)
        nh = 4 if b == 0 else 2
        w = 32 // nh
        for h in range(nh):
            nc.gpsimd.dma_start(
                out=fnat[:, h * w:(h + 1) * w, :],
                in_=fsrc[:, h * w:(h + 1) * w, :],
            )
        fnat_t[b] = fnat

    def qchain(b):
        """F_a.T -> Q.T -> Q~T (bf16, tiny)."""
        fa = fa_t.pop(b)
        fat = qpool.tile([128, 2, T], BF16, tag="fat", name="fat")  # [d_l, d_tile, t]
        for k in range(2):  # d tile
            pj = psum_pj.tile([128, T], BF16, tag="pj", name="pj")
            for m in range(2):  # t tile
                nc.tensor.matmul(
                    pj[:, m * 128:(m + 1) * 128],
                    fa[:, m, k * 128:(k + 1) * 128],
                    ident16[:],
                    is_transpose=True,
                    start=(m == 0),
                    stop=(m == 1),
                )
            nc.vector.tensor_copy(out=fat[:, k, :], in_=pj[:])

        qt = qpool.tile([128, 2, T], BF16, tag="qt", name="qt")  # [do_l, do_tile, t]
        for m in range(2):  # do tile
            pj = psum_pj.tile([128, T], F32, tag="pj", name="pj")
            for k in range(2):  # di tile
                nc.tensor.matmul(
                    pj[:],
                    wqt[:, k, m * 128:(m + 1) * 128],
                    fat[:, k, :],
                    start=(k == 0),
                    stop=(k == 1),
                )
            nc.vector.tensor_scalar_add(
                out=qt[:, m, :], in0=pj[:], scalar1=bq_sb[:, m:m + 1]
            )

        qct = qpool.tile([128, 2, T], BF16, tag="qct", name="qct")  # [i_l, i_tile, t]
        for m in range(2):  # i tile
            pj = psum_pj.tile([128, T], F32, tag="pj", name="pj")
            for k in range(2):  # j tile
                nc.tensor.matmul(
                    pj[:],
                    wk_sb[:, k, m * 128:(m + 1) * 128],
                    qt[:, k, :],
                    start=(k == 0),
                    stop=(k == 1),
                )
            nc.vector.tensor_scalar_mul(out=qct[:, m, :], in0=pj[:], scalar1=SCALE)
        qct_t[b] = qct

    def transpose_octet(b, ci, o):
        """8 PE transposes of [128,128] bf16 into one PSUM bank, one eviction."""
        fnat = fnat_t[b]
        fst = fst_t[b]
        pt = psum_tr.tile([128, 8, 128], BF16, tag="pt", name="pt")
        for k in range(8):
            sh = o * 8 + k
            nc.tensor.matmul(
                pt[:, k, :],
                fnat[:, sh, ci * 128:(ci + 1) * 128],
                ident16[:],
                is_transpose=True,
                start=(k == 0),
                stop=(k == 7),
            )
        nc.vector.tensor_copy(
            out=fst[:, ci, o * 1024:(o + 1) * 1024],
            in_=pt[:].rearrange("p a b -> p (a b)"),
        )

    def qk_chunk(b, tt, ck, p_tile, st):
        """QK + mask for one [128, 1024] chunk (2 PSUM banks), then exp."""
        fst = fst_t[b]
        qct = qct_t[b]
        pq = psum_qk.tile([128, 1024], F32, tag="pq", name="pq")
        for h in range(2):  # 512-wide half = one PSUM bank
            s0 = ck * 1024 + h * 512
            for ci in range(2):
                nc.tensor.matmul(
                    pq[:, h * 512:(h + 1) * 512],
                    qct[:, ci, tt * 128:(tt + 1) * 128],
                    fst[:, ci, s0:s0 + 512],
                    start=(ci == 0),
                    stop=False,
                )
            nc.tensor.matmul(
                pq[:, h * 512:(h + 1) * 512],
                ones16[:],
                mb_sb[:, b * HW + s0: b * HW + s0 + 512],
                start=False,
                stop=True,
            )
        nc.scalar.activation(
            out=p_tile[:, ck * 1024:(ck + 1) * 1024],
            in_=pq[:],
            func=mybir.ActivationFunctionType.Exp,
            accum_out=st[:, ck:ck + 1],
        )

    def finish_rowtile(b, tt, p_tile, st):
        rowsum = stats.tile([128, 1], F32, tag="rowsum", name="rowsum")
        nc.vector.reduce_sum(out=rowsum[:], in_=st[:], axis=mybir.AxisListType.X)
        recip = stats.tile([128, 1], F32, tag="recip", name="recip")
        nc.vector.reciprocal(out=recip[:], in_=rowsum[:])
        # normalize to bf16 staging + store, split in quarters for finer overlap
        s_tile = spool.tile([128, HW], BF16, tag="s", name="s")
        for h in range(2):
            sl = slice(h * (HW // 2), (h + 1) * (HW // 2))
            nc.vector.tensor_scalar_mul(
                out=s_tile[:, sl], in0=p_tile[:, sl], scalar1=recip[:, 0:1]
            )
            nc.sync.dma_start(
                out=S[b, tt * 128:(tt + 1) * 128, sl], in_=s_tile[:, sl]
            )

    # ---- software pipeline ----
    load_batch(0)
    qchain(0)
    OCTETS = [(ci, o) for ci in range(2) for o in range(4)]
    fst_t[0] = fst_pool.tile([128, 2, HW], BF16, tag="fst", name="fst")
    for ci, o in OCTETS:
        transpose_octet(0, ci, o)
    load_batch(1)
    qchain(1)

    for b in range(BS):
        if b + 2 < BS:
            load_batch(b + 2)
        if b + 1 < BS:
            fst_t[b + 1] = fst_pool.tile([128, 2, HW], BF16, tag="fst", name="fst")
        oi = 0
        for tt in range(2):
            p_tile = ppool.tile([128, HW], F32, tag="p", name="p")
            st = stats.tile([128, 4], F32, tag="st", name="st")
            for ck in range(4):
                qk_chunk(b, tt, ck, p_tile, st)
                if b + 1 < BS:
                    transpose_octet(b + 1, *OCTETS[oi])
                    oi += 1
            finish_rowtile(b, tt, p_tile, st)
        fnat_t.pop(b, None)
        fst_t.pop(b, None)
        qct_t.pop(b, None)
        if b + 2 < BS:
            qchain(b + 2)


def build_nc():
    nc = bacc.Bacc(
        "TRN2",
        target_bir_lowering=False,
        debug=False,
        num_devices=N_CORES,
    )
    F_a = nc.dram_tensor("F_a", [BS, T, D], F32, kind="ExternalInput")
    F_s = nc.dram_tensor("F_s", [BS, HW, D], F32, kind="ExternalInput")
    mbig = nc.dram_tensor("mbig", [BS, HW], BF16, kind="ExternalInput")
    Wq = nc.dram_tensor("Wq", [D, D], F32, kind="ExternalInput")
    Wk = nc.dram_tensor("Wk", [D, D], F32, kind="ExternalInput")
    bq = nc.dram_tensor("bq", [D], F32, kind="ExternalInput")
    S = nc.dram_tensor("S", [BS, T, HW], BF16, kind="ExternalOutput")

    with tile.TileContext(nc) as tc, ExitStack() as ctx:
        _build_body(
            tc, ctx, F_a.ap(), F_s.ap(), mbig.ap(), Wq.ap(), Wk.ap(), bq.ap(), S.ap()
        )
    nc.compile()
    return nc


def make_in_maps(F_a, F_s, M_s, Wq, bq, Wk):
    F_a = np.asarray(F_a, dtype=np.float32)
    F_s = np.asarray(F_s, dtype=np.float32)
    M_s = np.asarray(M_s)
    Wq = np.ascontiguousarray(np.asarray(Wq, dtype=np.float32))
    Wk = np.ascontiguousarray(np.asarray(Wk, dtype=np.float32))
    bq = np.ascontiguousarray(np.asarray(bq, dtype=np.float32))

    m = M_s.reshape(M_s.shape[0], -1) == 1  # [B, HW]
    mbig = np.where(m, np.float32(0.0), np.float32(MASK_NEG)).astype(
        ml_dtypes.bfloat16
    )

    in_maps = []
    for i in range(N_CORES):
        sl = slice(i * BS, (i + 1) * BS)
        in_maps.append(
            dict(
                F_a=np.ascontiguousarray(F_a[sl]),
                F_s=np.ascontiguousarray(F_s[sl]),
                mbig=np.ascontiguousarray(mbig[sl]),
                Wq=Wq,
                Wk=Wk,
                bq=bq,
            )
        )
    return in_maps


_NC_CACHE = None


def _get_nc():
    global _NC_CACHE
    if _NC_CACHE is None:
        _NC_CACHE = build_nc()
    return _NC_CACHE


def run(in_maps, **kwargs):
    from concourse import bass_utils

    nc = _get_nc()
    res = bass_utils.run_bass_kernel_spmd(
        nc, in_maps, core_ids=list(range(N_CORES)), **kwargs
    )
    return res


def kernel(F_a, F_s, M_s, Wq, bq, Wk, bk):
    in_maps = make_in_maps(F_a, F_s, M_s, Wq, bq, Wk)
    res = run(in_maps)
    return np.concatenate(
        [np.asarray(r["S"]).astype(np.float32) for r in res.results], axis=0
    )


# revision 23
# speedup vs baseline: 1.0137x; 1.0137x over previous
"""Trainium2 Bass kernel for masked spatial attention softmax.

Computes S = softmax((F_a@Wq.T + bq) @ (F_s@Wk.T + bk).T / sqrt(d) + mask)
over 8 NeuronCores, data-parallel over batch.

Key algebraic restructure: QK = Q_a @ K_s.T = ((F_a@Wq.T + bq) @ Wk) @ F_s.T
+ (Q_a . bk) 1^T.  The bk term is constant along the softmax axis, so it
drops out of the softmax entirely; K_s is never materialized.  This halves
the matmul FLOPs and removes a 4096xd projection per batch.

Software pipeline: batch b's QK/exp phase interleaves batch b+1's F_s
transposes on the PE stream (keeps PE dense, HAM warm); loads prefetch
two batches ahead.
"""

import math
from contextlib import ExitStack

import numpy as np
import ml_dtypes

import concourse.bass as bass
import concourse.tile as tile
from concourse import bacc, mybir
from concourse.masks import make_identity

# Problem shapes (hardcoded per contract; spec: B=32, T=256, HW=4096, d=256)
B_FULL = 32
N_CORES = 8
BS = B_FULL // N_CORES  # batches per core
T = 256
HW = 4096
D = 256
SCALE = 1.0 / math.sqrt(D)  # 1/16
MASK_NEG = -80.0  # exp(-80 + max_logit) << 1e-30; stays in ACT exp valid range

F32 = mybir.dt.float32
BF16 = mybir.dt.bfloat16


def _build_body(tc, ctx, F_a, F_s, mbig, Wq, Wk, bq, S):
    nc = tc.nc

    singles = ctx.enter_context(tc.tile_pool(name="singles", bufs=1))
    fnat_pool = ctx.enter_context(tc.tile_pool(name="fnat", bufs=2))
    fst_pool = ctx.enter_context(tc.tile_pool(name="fst", bufs=2))
    qpool = ctx.enter_context(tc.tile_pool(name="qpool", bufs=2))
    ppool = ctx.enter_context(tc.tile_pool(name="ppool", bufs=3))
    spool = ctx.enter_context(tc.tile_pool(name="spool", bufs=2))
    stats = ctx.enter_context(tc.tile_pool(name="stats", bufs=4))
    psum_tr = ctx.enter_context(tc.tile_pool(name="psum_tr", bufs=1, space="PSUM"))
    psum_qk = ctx.enter_context(tc.tile_pool(name="psum_qk", bufs=3, space="PSUM"))
    psum_pj = ctx.enter_context(tc.tile_pool(name="psum_pj", bufs=1, space="PSUM"))

    # ---- constants ----
    ident16 = singles.tile([128, 128], BF16, tag="ident16", name="ident16")
    make_identity(nc, ident16[:])
    ones16 = singles.tile([1, 128], BF16, tag="ones16", name="ones16")
    nc.vector.memset(ones16[:], 1.0)

    # mask additive term, all batches: [1, BS*HW] bf16 (0 or MASK_NEG)
    mb_sb = singles.tile([1, BS * HW], BF16, tag="mb", name="mb")
    nc.sync.dma_start(out=mb_sb[:], in_=mbig.rearrange("b s -> (b s)")[None, :])

    # bq as per-partition scalars: [128, 2] (do-tile major in free dim)
    bq_sb = singles.tile([128, 2], F32, tag="bq", name="bq")
    nc.sync.dma_start(out=bq_sb[:], in_=bq.rearrange("(a p) -> p a", p=128))

    # Wk natural: lhsT[j, i] for Q~T = sum_j Wk[j,i] Q.T[j,t]  (bf16 cast DMA)
    wk_sb = singles.tile([128, 2, D], BF16, tag="wk", name="wk")
    nc.gpsimd.dma_start(out=wk_sb[:], in_=Wk.rearrange("(jh jl) i -> jl jh i", jl=128))

    # Wq loaded natural (bf16), then PE-transposed to WqT[di, do]
    wq_nat = singles.tile([128, 2, D], BF16, tag="wqn", name="wqn")
    nc.gpsimd.dma_start(
        out=wq_nat[:], in_=Wq.rearrange("(oh ol) i -> ol oh i", ol=128)
    )
    wqt = singles.tile([128, 2, D], BF16, tag="wqt", name="wqt")
    for k in range(2):  # di tile
        pj = psum_pj.tile([128, D], BF16, tag="pj", name="pj")
        for m in range(2):  # do tile
            nc.tensor.matmul(
                pj[:, m * 128:(m + 1) * 128],
                wq_nat[:, m, k * 128:(k + 1) * 128],
                ident16[:],
                is_transpose=True,
                start=(m == 0),
                stop=(m == 1),
            )
        nc.vector.tensor_copy(out=wqt[:, k, :], in_=pj[:])

    fa_t, fnat_t, fst_t, qct_t = {}, {}, {}, {}

    def load_batch(b):
        """Prefetch F_a[b] (small, first) and F_s[b] in halves (SWDGE casts)."""
        fa = qpool.tile([128, 2, D], BF16, tag="fa", name="fa")  # [tl, th, d]
        nc.gpsimd.dma_start(
            out=fa[:], in_=F_a[b].rearrange("(th tl) d -> tl th d", tl=128)
        )
        fa_t[b] = fa
        fnat = fnat_pool.tile([128, 32, D], BF16, tag="fnat", name="fnat")  # [sl, sh, c]
        fsrc = F_s[b].rearrange("(sh sl) c -> sl sh c", sl=128)
        nh = 4 if b == 0 else 2
        w = 32 // nh
        for h in range(nh):
            nc.gpsimd.dma_start(
                out=fnat[:, h * w:(h + 1) * w, :],
                in_=fsrc[:, h * w:(h + 1) * w, :],
            )
        fnat_t[b] = fnat

    def qchain(b):
        """F_a.T -> Q.T -> Q~T (bf16, tiny)."""
        fa = fa_t.pop(b)
        fat = qpool.tile([128, 2, T], BF16, tag="fat", name="fat")  # [d_l, d_tile, t]
        for k in range(2):  # d tile
            pj = psum_pj.tile([128, T], BF16, tag="pj", name="pj")
            for m in range(2):  # t tile
                nc.tensor.matmul(
                    pj[:, m * 128:(m + 1) * 128],
                    fa[:, m, k * 128:(k + 1) * 128],
                    ident16[:],
                    is_transpose=True,
                    start=(m == 0),
                    stop=(m == 1),
                )
            nc.vector.tensor_copy(out=fat[:, k, :], in_=pj[:])

        qt = qpool.tile([128, 2, T], BF16, tag="qt", name="qt")  # [do_l, do_tile, t]
        for m in range(2):  # do tile
            pj = psum_pj.tile([128, T], F32, tag="pj", name="pj")
            for k in range(2):  # di tile
                nc.tensor.matmul(
                    pj[:],
                    wqt[:, k, m * 128:(m + 1) * 128],
                    fat[:, k, :],
                    start=(k == 0),
                    stop=(k == 1),
                )
            nc.vector.tensor_scalar_add(
                out=qt[:, m, :], in0=pj[:], scalar1=bq_sb[:, m:m + 1]
            )

        qct = qpool.tile([128, 2, T], BF16, tag="qct", name="qct")  # [i_l, i_tile, t]
        for m in range(2):  # i tile
            pj = psum_pj.tile([128, T], F32, tag="pj", name="pj")
            for k in range(2):  # j tile
                nc.tensor.matmul(
                    pj[:],
                    wk_sb[:, k, m * 128:(m + 1) * 128],
                    qt[:, k, :],
                    start=(k == 0),
                    stop=(k == 1),
                )
            nc.vector.tensor_scalar_mul(out=qct[:, m, :], in0=pj[:], scalar1=SCALE)
        qct_t[b] = qct

    def transpose_octet(b, ci, o):
        """8 PE transposes of [128,128] bf16 into one PSUM bank, one eviction."""
        fnat = fnat_t[b]
        fst = fst_t[b]
        pt = psum_tr.tile([128, 8, 128], BF16, tag="pt", name="pt")
        for k in range(8):
            sh = o * 8 + k
            nc.tensor.matmul(
                pt[:, k, :],
                fnat[:, sh, ci * 128:(ci + 1) * 128],
                ident16[:],
                is_transpose=True,
                start=(k == 0),
                stop=(k == 7),
            )
        nc.vector.tensor_copy(
            out=fst[:, ci, o * 1024:(o + 1) * 1024],
            in_=pt[:].rearrange("p a b -> p (a b)"),
        )

    def qk_chunk(b, tt, ck, p_tile, st):
        """QK + mask for one [128, 1024] chunk (2 PSUM banks), then exp."""
        fst = fst_t[b]
        qct = qct_t[b]
        pq = psum_qk.tile([128, 1024], F32, tag="pq", name="pq")
        for h in range(2):  # 512-wide half = one PSUM bank
            s0 = ck * 1024 + h * 512
            for ci in range(2):
                nc.tensor.matmul(
                    pq[:, h * 512:(h + 1) * 512],
                    qct[:, ci, tt * 128:(tt + 1) * 128],
                    fst[:, ci, s0:s0 + 512],
                    start=(ci == 0),
                    stop=False,
                )
            nc.tensor.matmul(
                pq[:, h * 512:(h + 1) * 512],
                ones16[:],
                mb_sb[:, b * HW + s0: b * HW + s0 + 512],
                start=False,
                stop=True,
            )
        nc.scalar.activation(
            out=p_tile[:, ck * 1024:(ck + 1) * 1024],
            in_=pq[:],
            func=mybir.ActivationFunctionType.Exp,
            accum_out=st[:, ck:ck + 1],
        )

    def finish_rowtile(b, tt, p_tile, st):
        rowsum = stats.tile([128, 1], F32, tag="rowsum", name="rowsum")
        nc.vector.reduce_sum(out=rowsum[:], in_=st[:], axis=mybir.AxisListType.X)
        recip = stats.tile([128, 1], F32, tag="recip", name="recip")
        nc.vector.reciprocal(out=recip[:], in_=rowsum[:])
        # normalize to bf16 staging + store, split in quarters for finer overlap
        s_tile = spool.tile([128, HW], BF16, tag="s", name="s")
        for h in range(2):
            sl = slice(h * (HW // 2), (h + 1) * (HW // 2))
            nc.vector.tensor_scalar_mul(
                out=s_tile[:, sl], in0=p_tile[:, sl], scalar1=recip[:, 0:1]
            )
            nc.sync.dma_start(
                out=S[b, tt * 128:(tt + 1) * 128, sl], in_=s_tile[:, sl]
            )

    # ---- software pipeline ----
    load_batch(0)
    qchain(0)
    OCTETS = [(ci, o) for ci in range(2) for o in range(4)]
    # Batch 0 prologue: o-major octet order, first two pairs up front, the
    # rest interleaved into batch-0 tt=0 chunks (QK can start ~8us earlier:
    # chunk ck only needs octet pairs <= ck).
    OCT0 = [(ci, o) for o in range(4) for ci in range(2)]
    fst_t[0] = fst_pool.tile([128, 2, HW], BF16, tag="fst", name="fst")
    for ci, o in OCT0[:4]:
        transpose_octet(0, ci, o)
    load_batch(1)
    qchain(1)

    for b in range(BS):
        if b + 2 < BS:
            load_batch(b + 2)
        if b + 1 < BS:
            fst_t[b + 1] = fst_pool.tile([128, 2, HW], BF16, tag="fst", name="fst")
        oi = 0
        for tt in range(2):
            p_tile = ppool.tile([128, HW], F32, tag="p", name="p")
            st = stats.tile([128, 4], F32, tag="st", name="st")
            for ck in range(4):
                if b == 0 and tt == 0 and ck >= 2:
                    # finish batch-0's own transposes just in time
                    transpose_octet(0, *OCT0[2 * ck])
                    transpose_octet(0, *OCT0[2 * ck + 1])
                qk_chunk(b, tt, ck, p_tile, st)
                if b + 1 < BS and not (b == 0 and tt == 0):
                    n_emit = 2 if b == 0 else 1
                    for _ in range(n_emit):
                        if oi < 8:
                            transpose_octet(b + 1, *OCTETS[oi])
                            oi += 1
            finish_rowtile(b, tt, p_tile, st)
        fnat_t.pop(b, None)
        fst_t.pop(b, None)
        qct_t.pop(b, None)
        if b + 2 < BS:
            qchain(b + 2)


def build_nc():
    nc = bacc.Bacc(
        "TRN2",
        target_bir_lowering=False,
        debug=False,
        num_devices=N_CORES,
    )
    F_a = nc.dram_tensor("F_a", [BS, T, D], F32, kind="ExternalInput")
    F_s = nc.dram_tensor("F_s", [BS, HW, D], F32, kind="ExternalInput")
    mbig = nc.dram_tensor("mbig", [BS, HW], BF16, kind="ExternalInput")
    Wq = nc.dram_tensor("Wq", [D, D], F32, kind="ExternalInput")
    Wk = nc.dram_tensor("Wk", [D, D], F32, kind="ExternalInput")
    bq = nc.dram_tensor("bq", [D], F32, kind="ExternalInput")
    S = nc.dram_tensor("S", [BS, T, HW], BF16, kind="ExternalOutput")

    with tile.TileContext(nc) as tc, ExitStack() as ctx:
        _build_body(
            tc, ctx, F_a.ap(), F_s.ap(), mbig.ap(), Wq.ap(), Wk.ap(), bq.ap(), S.ap()
        )
    nc.compile()
    return nc


def make_in_maps(F_a, F_s, M_s, Wq, bq, Wk):
    F_a = np.asarray(F_a, dtype=np.float32)
    F_s = np.asarray(F_s, dtype=np.float32)
    M_s = np.asarray(M_s)
    Wq = np.ascontiguousarray(np.asarray(Wq, dtype=np.float32))
    Wk = np.ascontiguousarray(np.asarray(Wk, dtype=np.float32))
    bq = np.ascontiguousarray(np.asarray(bq, dtype=np.float32))

    m = M_s.reshape(M_s.shape[0], -1) == 1  # [B, HW]
    mbig = np.where(m, np.float32(0.0), np.float32(MASK_NEG)).astype(
        ml_dtypes.bfloat16
    )

    in_maps = []
    for i in range(N_CORES):
        sl = slice(i * BS, (i + 1) * BS)
        in_maps.append(
            dict(
                F_a=np.ascontiguousarray(F_a[sl]),
                F_s=np.ascontiguousarray(F_s[sl]),
                mbig=np.ascontiguousarray(mbig[sl]),
                Wq=Wq,
                Wk=Wk,
                bq=bq,
            )
        )
    return in_maps


_NC_CACHE = None


def _get_nc():
    global _NC_CACHE
    if _NC_CACHE is None:
        _NC_CACHE = build_nc()
    return _NC_CACHE


def run(in_maps, **kwargs):
    from concourse import bass_utils

    nc = _get_nc()
    res = bass_utils.run_bass_kernel_spmd(
        nc, in_maps, core_ids=list(range(N_CORES)), **kwargs
    )
    return res


def kernel(F_a, F_s, M_s, Wq, bq, Wk, bk):
    in_maps = make_in_maps(F_a, F_s, M_s, Wq, bq, Wk)
    res = run(in_maps)
    return np.concatenate(
        [np.asarray(r["S"]).astype(np.float32) for r in res.results], axis=0
    )


# revision 24
# speedup vs baseline: 1.2119x; 1.1955x over previous
"""Trainium2 Bass kernel for masked spatial attention softmax.

Computes S = softmax((F_a@Wq.T + bq) @ (F_s@Wk.T + bk).T / sqrt(d) + mask)
over 8 NeuronCores, data-parallel over batch.

Key algebraic restructure: QK = Q_a @ K_s.T = ((F_a@Wq.T + bq) @ Wk) @ F_s.T
+ (Q_a . bk) 1^T.  The bk term is constant along the softmax axis, so it
drops out of the softmax entirely; K_s is never materialized.  This halves
the matmul FLOPs and removes a 4096xd projection per batch.

Software pipeline: batch b's QK/exp phase interleaves batch b+1's F_s
transposes on the PE stream (keeps PE dense, HAM warm); loads prefetch
two batches ahead.
"""

import math
from contextlib import ExitStack

import numpy as np
import ml_dtypes

import concourse.bass as bass
import concourse.tile as tile
from concourse import bacc, mybir
from concourse.masks import make_identity

# Problem shapes (hardcoded per contract; spec: B=32, T=256, HW=4096, d=256)
B_FULL = 32
N_CORES = 8
BS = B_FULL // N_CORES  # batches per core
T = 256
HW = 4096
D = 256
SCALE = 1.0 / math.sqrt(D)  # 1/16
MASK_NEG = -80.0  # exp(-80 + max_logit) << 1e-30; stays in ACT exp valid range

F32 = mybir.dt.float32
BF16 = mybir.dt.bfloat16


def _build_body(tc, ctx, F_a, F_s, mbig, Wq, Wk, bq, S):
    nc = tc.nc

    singles = ctx.enter_context(tc.tile_pool(name="singles", bufs=1))
    fnat_pool = ctx.enter_context(tc.tile_pool(name="fnat", bufs=2))
    fst_pool = ctx.enter_context(tc.tile_pool(name="fst", bufs=2))
    qpool = ctx.enter_context(tc.tile_pool(name="qpool", bufs=2))
    ppool = ctx.enter_context(tc.tile_pool(name="ppool", bufs=3))
    spool = ctx.enter_context(tc.tile_pool(name="spool", bufs=2))
    stats = ctx.enter_context(tc.tile_pool(name="stats", bufs=4))
    psum_tr = ctx.enter_context(tc.tile_pool(name="psum_tr", bufs=1, space="PSUM"))
    psum_qk = ctx.enter_context(tc.tile_pool(name="psum_qk", bufs=3, space="PSUM"))
    psum_pj = ctx.enter_context(tc.tile_pool(name="psum_pj", bufs=1, space="PSUM"))

    # ---- constants ----
    ident16 = singles.tile([128, 128], BF16, tag="ident16", name="ident16")
    make_identity(nc, ident16[:])
    ones16 = singles.tile([1, 128], BF16, tag="ones16", name="ones16")
    nc.vector.memset(ones16[:], 1.0)

    # mask additive term, all batches: [1, BS*HW] bf16 (0 or MASK_NEG)
    mb_sb = singles.tile([1, BS * HW], BF16, tag="mb", name="mb")
    nc.sync.dma_start(out=mb_sb[:], in_=mbig.rearrange("b s -> (b s)")[None, :])

    # bq as per-partition scalars: [128, 2] (do-tile major in free dim)
    bq_sb = singles.tile([128, 2], F32, tag="bq", name="bq")
    nc.sync.dma_start(out=bq_sb[:], in_=bq.rearrange("(a p) -> p a", p=128))

    # Wk natural: lhsT[j, i] for Q~T = sum_j Wk[j,i] Q.T[j,t]  (bf16 cast DMA)
    wk_sb = singles.tile([128, 2, D], BF16, tag="wk", name="wk")
    nc.gpsimd.dma_start(out=wk_sb[:], in_=Wk.rearrange("(jh jl) i -> jl jh i", jl=128))

    # Wq loaded natural (bf16), then PE-transposed to WqT[di, do]
    wq_nat = singles.tile([128, 2, D], BF16, tag="wqn", name="wqn")
    nc.gpsimd.dma_start(
        out=wq_nat[:], in_=Wq.rearrange("(oh ol) i -> ol oh i", ol=128)
    )
    wqt = singles.tile([128, 2, D], BF16, tag="wqt", name="wqt")
    for k in range(2):  # di tile
        pj = psum_pj.tile([128, D], BF16, tag="pj", name="pj")
        for m in range(2):  # do tile
            nc.tensor.matmul(
                pj[:, m * 128:(m + 1) * 128],
                wq_nat[:, m, k * 128:(k + 1) * 128],
                ident16[:],
                is_transpose=True,
                start=(m == 0),
                stop=(m == 1),
            )
        nc.vector.tensor_copy(out=wqt[:, k, :], in_=pj[:])

    fa_t, fnat_t, fst_t, qct_t = {}, {}, {}, {}

    def load_batch(b):
        """Prefetch F_a[b] (small, first) and F_s[b] in halves (SWDGE casts)."""
        fa = qpool.tile([128, 2, D], BF16, tag="fa", name="fa")  # [tl, th, d]
        nc.gpsimd.dma_start(
            out=fa[:], in_=F_a[b].rearrange("(th tl) d -> tl th d", tl=128)
        )
        fa_t[b] = fa
        fnat = fnat_pool.tile([128, 32, D], BF16, tag="fnat", name="fnat")  # [sl, sh, c]
        fsrc = F_s[b].rearrange("(sh sl) c -> sl sh c", sl=128)
        nh = 4 if b == 0 else 2
        w = 32 // nh
        for h in range(nh):
            nc.gpsimd.dma_start(
                out=fnat[:, h * w:(h + 1) * w, :],
                in_=fsrc[:, h * w:(h + 1) * w, :],
            )
        fnat_t[b] = fnat

    def qchain(b):
        """F_a.T -> Q.T -> Q~T (bf16, tiny)."""
        fa = fa_t.pop(b)
        fat = qpool.tile([128, 2, T], BF16, tag="fat", name="fat")  # [d_l, d_tile, t]
        for k in range(2):  # d tile
            pj = psum_pj.tile([128, T], BF16, tag="pj", name="pj")
            for m in range(2):  # t tile
                nc.tensor.matmul(
                    pj[:, m * 128:(m + 1) * 128],
                    fa[:, m, k * 128:(k + 1) * 128],
                    ident16[:],
                    is_transpose=True,
                    start=(m == 0),
                    stop=(m == 1),
                )
            nc.vector.tensor_copy(out=fat[:, k, :], in_=pj[:])

        qt = qpool.tile([128, 2, T], BF16, tag="qt", name="qt")  # [do_l, do_tile, t]
        for m in range(2):  # do tile
            pj = psum_pj.tile([128, T], F32, tag="pj", name="pj")
            for k in range(2):  # di tile
                nc.tensor.matmul(
                    pj[:],
                    wqt[:, k, m * 128:(m + 1) * 128],
                    fat[:, k, :],
                    start=(k == 0),
                    stop=(k == 1),
                )
            nc.vector.tensor_scalar_add(
                out=qt[:, m, :], in0=pj[:], scalar1=bq_sb[:, m:m + 1]
            )

        qct = qpool.tile([128, 2, T], BF16, tag="qct", name="qct")  # [i_l, i_tile, t]
        for m in range(2):  # i tile
            pj = psum_pj.tile([128, T], F32, tag="pj", name="pj")
            for k in range(2):  # j tile
                nc.tensor.matmul(
                    pj[:],
                    wk_sb[:, k, m * 128:(m + 1) * 128],
                    qt[:, k, :],
                    start=(k == 0),
                    stop=(k == 1),
                )
            nc.vector.tensor_scalar_mul(out=qct[:, m, :], in0=pj[:], scalar1=SCALE)
        qct_t[b] = qct

    def transpose_octet(b, ci, o):
        """8 PE transposes of [128,128] bf16 into one PSUM bank, one eviction."""
        fnat = fnat_t[b]
        fst = fst_t[b]
        pt = psum_tr.tile([128, 8, 128], BF16, tag="pt", name="pt")
        for k in range(8):
            sh = o * 8 + k
            nc.tensor.matmul(
                pt[:, k, :],
                fnat[:, sh, ci * 128:(ci + 1) * 128],
                ident16[:],
                is_transpose=True,
                start=(k == 0),
                stop=(k == 7),
            )
        nc.vector.tensor_copy(
            out=fst[:, ci, o * 1024:(o + 1) * 1024],
            in_=pt[:].rearrange("p a b -> p (a b)"),
        )

    def qk_chunk(b, tt, ck, p_tile, st):
        """QK + mask for one [128, 1024] chunk (2 PSUM banks), then exp."""
        fst = fst_t[b]
        qct = qct_t[b]
        pq = psum_qk.tile([128, 1024], F32, tag="pq", name="pq")
        # weight-reuse ordering: both banks' matmuls grouped by lhsT
        for ci in range(2):
            for h in range(2):  # 512-wide half = one PSUM bank
                s0 = ck * 1024 + h * 512
                nc.tensor.matmul(
                    pq[:, h * 512:(h + 1) * 512],
                    qct[:, ci, tt * 128:(tt + 1) * 128],
                    fst[:, ci, s0:s0 + 512],
                    start=(ci == 0),
                    stop=False,
                )
        for h in range(2):
            s0 = ck * 1024 + h * 512
            nc.tensor.matmul(
                pq[:, h * 512:(h + 1) * 512],
                ones16[:],
                mb_sb[:, b * HW + s0: b * HW + s0 + 512],
                start=False,
                stop=True,
            )
        nc.scalar.activation(
            out=p_tile[:, ck * 1024:(ck + 1) * 1024],
            in_=pq[:],
            func=mybir.ActivationFunctionType.Exp,
            accum_out=st[:, ck:ck + 1],
        )

    def finish_rowtile(b, tt, p_tile, st):
        rowsum = stats.tile([128, 1], F32, tag="rowsum", name="rowsum")
        nc.vector.reduce_sum(out=rowsum[:], in_=st[:], axis=mybir.AxisListType.X)
        recip = stats.tile([128, 1], F32, tag="recip", name="recip")
        nc.vector.reciprocal(out=recip[:], in_=rowsum[:])
        # normalize to bf16 staging + store, split in quarters for finer overlap
        s_tile = spool.tile([128, HW], BF16, tag="s", name="s")
        for h in range(2):
            sl = slice(h * (HW // 2), (h + 1) * (HW // 2))
            nc.vector.tensor_scalar_mul(
                out=s_tile[:, sl], in0=p_tile[:, sl], scalar1=recip[:, 0:1]
            )
            nc.sync.dma_start(
                out=S[b, tt * 128:(tt + 1) * 128, sl], in_=s_tile[:, sl]
            )

    # ---- software pipeline ----
    load_batch(0)
    qchain(0)
    OCTETS = [(ci, o) for ci in range(2) for o in range(4)]
    # Batch 0 prologue: o-major octet order, first two pairs up front, the
    # rest interleaved into batch-0 tt=0 chunks (QK can start ~8us earlier:
    # chunk ck only needs octet pairs <= ck).
    OCT0 = [(ci, o) for o in range(4) for ci in range(2)]
    fst_t[0] = fst_pool.tile([128, 2, HW], BF16, tag="fst", name="fst")
    for ci, o in OCT0[:4]:
        transpose_octet(0, ci, o)
    load_batch(1)
    qchain(1)

    for b in range(BS):
        if b + 2 < BS:
            load_batch(b + 2)
        if b + 1 < BS:
            fst_t[b + 1] = fst_pool.tile([128, 2, HW], BF16, tag="fst", name="fst")
        oi = 0
        for tt in range(2):
            p_tile = ppool.tile([128, HW], F32, tag="p", name="p")
            st = stats.tile([128, 4], F32, tag="st", name="st")
            for ck in range(4):
                if b == 0 and tt == 0 and ck >= 2:
                    # finish batch-0's own transposes just in time
                    transpose_octet(0, *OCT0[2 * ck])
                    transpose_octet(0, *OCT0[2 * ck + 1])
                qk_chunk(b, tt, ck, p_tile, st)
                if b + 1 < BS and not (b == 0 and tt == 0):
                    n_emit = 2 if b == 0 else 1
                    for _ in range(n_emit):
                        if oi < 8:
                            transpose_octet(b + 1, *OCTETS[oi])
                            oi += 1
            finish_rowtile(b, tt, p_tile, st)
        fnat_t.pop(b, None)
        fst_t.pop(b, None)
        qct_t.pop(b, None)
        if b + 2 < BS:
            qchain(b + 2)


def build_nc():
    nc = bacc.Bacc(
        "TRN2",
        target_bir_lowering=False,
        debug=False,
        num_devices=N_CORES,
    )
    F_a = nc.dram_tensor("F_a", [BS, T, D], F32, kind="ExternalInput")
    F_s = nc.dram_tensor("F_s", [BS, HW, D], F32, kind="ExternalInput")
    mbig = nc.dram_tensor("mbig", [BS, HW], BF16, kind="ExternalInput")
    Wq = nc.dram_tensor("Wq", [D, D], F32, kind="ExternalInput")
    Wk = nc.dram_tensor("Wk", [D, D], F32, kind="ExternalInput")
    bq = nc.dram_tensor("bq", [D], F32, kind="ExternalInput")
    S = nc.dram_tensor("S", [BS, T, HW], BF16, kind="ExternalOutput")

    with tile.TileContext(nc) as tc, ExitStack() as ctx:
        _build_body(
            tc, ctx, F_a.ap(), F_s.ap(), mbig.ap(), Wq.ap(), Wk.ap(), bq.ap(), S.ap()
        )
    nc.compile()
    return nc


def make_in_maps(F_a, F_s, M_s, Wq, bq, Wk):
    F_a = np.asarray(F_a, dtype=np.float32)
    F_s = np.asarray(F_s, dtype=np.float32)
    M_s = np.asarray(M_s)
    Wq = np.ascontiguousarray(np.asarray(Wq, dtype=np.float32))
    Wk = np.ascontiguousarray(np.asarray(Wk, dtype=np.float32))
    bq = np.ascontiguousarray(np.asarray(bq, dtype=np.float32))

    m = M_s.reshape(M_s.shape[0], -1) == 1  # [B, HW]
    mbig = np.where(m, np.float32(0.0), np.float32(MASK_NEG)).astype(
        ml_dtypes.bfloat16
    )

    in_maps = []
    for i in range(N_CORES):
        sl = slice(i * BS, (i + 1) * BS)
        in_maps.append(
            dict(
                F_a=np.ascontiguousarray(F_a[sl]),
                F_s=np.ascontiguousarray(F_s[sl]),
                mbig=np.ascontiguousarray(mbig[sl]),
                Wq=Wq,
                Wk=Wk,
                bq=bq,
            )
        )
    return in_maps


_NC_CACHE = None


def _get_nc():
    global _NC_CACHE
    if _NC_CACHE is None:
        _NC_CACHE = build_nc()
    return _NC_CACHE


def run(in_maps, **kwargs):
    from concourse import bass_utils

    nc = _get_nc()
    res = bass_utils.run_bass_kernel_spmd(
        nc, in_maps, core_ids=list(range(N_CORES)), **kwargs
    )
    return res


def kernel(F_a, F_s, M_s, Wq, bq, Wk, bk):
    in_maps = make_in_maps(F_a, F_s, M_s, Wq, bq, Wk)
    res = run(in_maps)
    return np.concatenate(
        [np.asarray(r["S"]).astype(np.float32) for r in res.results], axis=0
    )
